# revision 1
# baseline (speedup 1.0000x reference)
"""GAT model as a single Bass/Tile SPMD program for 8 TRN2 NeuronCores.

Reference: 3x (GATConv(4 heads x 32) + BatchNorm + ELU) + GATConv(1x64)
+ global mean pool over 64 graphs.

Sharding: 1D dst-node partition (nloc nodes/core); each core owns the
incident edges of its dst nodes. Per layer:
  transform (feature-major): h^T = W^T @ y^T, attention logits via
    precomposed [W@as_blk | W@ad_blk]; transpose into node-major gather
    tables hcat (row = [h | als | pad]) and aldrep (row = [ald | pad]);
    AllGather hcat so all src-gathers are core-local.
  aggregation: dma_gather hcat rows by src (int16 split in two halves),
    aldrep rows by dst_local; e = lrelu(als+ald); ex = exp(e) (logits are
    bounded so no max-subtraction is needed); msg = h*ex; one-hot matmul
    S^T @ [msg|ex] accumulates per-128-node-block psum = [sum(ex*h)|sum(ex)].
BatchNorm stats via a 1KB AllReduce; bias before BatchNorm cancels exactly
(mean subtraction), so b0..b2 are dropped; b3 and the pool division are
applied on the host to the per-core pooled partial sums.

The edge structure is static input data, preprocessed on the host into
index tables, so the instruction stream is identical on all cores
(per-(block,region) tile counts padded to the max across cores).
"""
from dataclasses import dataclass, field

import numpy as np

import concourse.bacc as bacc
import concourse.tile as tile
from concourse import mybir
from concourse.masks import make_identity

F32 = mybir.dt.float32
BF16 = mybir.dt.bfloat16
I16 = mybir.dt.int16
AF = mybir.ActivationFunctionType
ALU = mybir.AluOpType

P = 128


@dataclass
class GATConfig:
    n: int = 50000
    e: int = 800000
    f_in: int = 128
    hid: int = 32
    heads: int = 4
    out_c: int = 64
    ng: int = 64
    n_cores: int = 8
    chunk_tiles: int = 24
    eps_bn: float = 1e-5
    eps_sm: float = 1e-16
    half: int = 32768
    bf16: bool = True
    gbufs: int = 3
    ablate: str = ""  # comma tokens: noald,nohcat,nomm,nos,nomsg,nocc,noe

    @property
    def hd(self):
        return self.hid * self.heads

    @property
    def nloc(self):
        assert self.n % self.n_cores == 0
        return self.n // self.n_cores

    @property
    def nblk(self):
        return (self.nloc + P - 1) // P


@dataclass
class GraphMeta:
    cfg: GATConfig
    cap_l: list = field(default_factory=list)
    cap_h: list = field(default_factory=list)
    total_tiles: int = 0
    idx_lh: list = field(default_factory=list)
    aidx: list = field(default_factory=list)
    doff: list = field(default_factory=list)
    ptab: list = field(default_factory=list)
    cnt: np.ndarray | None = None
    perm: np.ndarray | None = None  # node v -> permuted position


def wrap_idx16(idx: np.ndarray) -> np.ndarray:
    n = len(idx)
    assert n % 16 == 0
    out = np.empty((16, n // 16), dtype=np.int16)
    out[:, :] = idx.astype(np.int16).reshape(n // 16, 16).T
    return np.tile(out, (8, 1))


def make_perm(cfg: GATConfig, edge_index: np.ndarray) -> np.ndarray:
    """Identity assignment. (Degree-balancing experiments did not reduce the
    per-(block,region) tile caps: the region split depends on the permuted
    source index, so the balance target is circular.)"""
    return np.arange(cfg.n, dtype=np.int64)


def preprocess(cfg: GATConfig, edge_index: np.ndarray, batch_ids: np.ndarray) -> GraphMeta:
    n, nc_, nloc, nblk = cfg.n, cfg.n_cores, cfg.nloc, cfg.nblk
    perm = make_perm(cfg, edge_index)
    src = np.concatenate([perm[np.asarray(edge_index[0], np.int64)],
                          np.arange(n, dtype=np.int64)])
    dst = np.concatenate([perm[np.asarray(edge_index[1], np.int64)],
                          np.arange(n, dtype=np.int64)])
    # self loops: permuted node v sits at position perm[v]
    src[len(edge_index[0]):] = perm[np.arange(n)]
    dst[len(edge_index[1]):] = perm[np.arange(n)]
    batch_ids = np.asarray(batch_ids, np.int64)

    core = dst // nloc
    dstl = dst - core * nloc
    blk = dstl // P
    region = (src >= cfg.half).astype(np.int64)

    ecount = np.zeros((nc_, nblk, 2), dtype=np.int64)
    order = np.lexsort((dstl, region, blk, core))
    src_s, dstl_s, blk_s, reg_s, core_s = (
        src[order], dstl[order], blk[order], region[order], core[order])
    np.add.at(ecount, (core_s, blk_s, reg_s), 1)

    tcnt = (ecount + P - 1) // P
    cap_l = np.maximum(tcnt[:, :, 0].max(axis=0), 1)
    cap_h = np.maximum(tcnt[:, :, 1].max(axis=0), 1)
    total_tiles = int(cap_l.sum() + cap_h.sum())

    meta = GraphMeta(cfg=cfg, cap_l=[int(v) for v in cap_l],
                     cap_h=[int(v) for v in cap_h], total_tiles=total_tiles)

    # slot start of each (block, region) group; identical across cores
    starts = np.zeros((nblk, 2), dtype=np.int64)
    off = 0
    for r in range(2):
        caps = cap_l if r == 0 else cap_h
        for b in range(nblk):
            starts[b, r] = off
            off += int(caps[b]) * P
    assert off == total_tiles * P

    nslot = total_tiles * P
    slot_src = np.zeros((nc_, nslot), dtype=np.int64)
    slot_dstl = np.zeros((nc_, nslot), dtype=np.int64)
    slot_doff = np.full((nc_, nslot), -1.0, dtype=np.float32)

    grp_key = (core_s * nblk + blk_s) * 2 + reg_s
    grp_change = np.empty(len(grp_key), dtype=bool)
    grp_change[0] = True
    grp_change[1:] = grp_key[1:] != grp_key[:-1]
    grp_first = np.where(grp_change)[0]
    reps = np.diff(np.append(grp_first, len(grp_key)))
    within = np.arange(len(grp_key)) - np.repeat(grp_first, reps)
    pos = starts[blk_s, reg_s] + within
    slot_src[core_s, pos] = src_s
    slot_dstl[core_s, pos] = dstl_s
    slot_doff[core_s, pos] = (dstl_s % P).astype(np.float32)

    hi_mask = np.zeros(nslot, dtype=bool)
    hi_mask[int(cap_l.sum()) * P:] = True

    for k in range(nc_):
        valid = slot_doff[k] >= 0
        gidx = np.where(valid & hi_mask, slot_src[k] - cfg.half,
                        np.where(valid, slot_src[k], 0))
        assert gidx.min() >= 0 and gidx.max() < cfg.half, (gidx.min(), gidx.max())
        meta.idx_lh.append(wrap_idx16(gidx))
        meta.aidx.append(wrap_idx16(np.where(valid, slot_dstl[k], 0)))
        d = np.empty((P, total_tiles), dtype=np.float32)
        d[:, :] = slot_doff[k].reshape(total_tiles, P).T
        meta.doff.append(d)

        pt = np.zeros((P, nblk * 64), dtype=np.float32)
        inv = np.empty(n, dtype=np.int64)
        inv[perm] = np.arange(n)
        for b in range(nblk):
            lo = k * nloc + b * P
            hi = min(k * nloc + (b + 1) * P, (k + 1) * nloc)
            ids = batch_ids[inv[lo:hi]]
            pt[np.arange(hi - lo), b * 64 + ids] = 1.0
        meta.ptab.append(pt)

    meta.cnt = np.bincount(batch_ids, minlength=cfg.ng).astype(np.float32)
    meta.perm = perm
    return meta


def make_as_block(a: np.ndarray) -> np.ndarray:
    h, c = a.shape
    out = np.zeros((h * c, h), dtype=np.float32)
    for i in range(h):
        out[i * c:(i + 1) * c, i] = a[i]
    return out


def build_inputs(cfg: GATConfig, meta: GraphMeta, inp: dict) -> list[dict]:
    nloc = cfg.nloc
    x = np.asarray(inp["x"], np.float32)
    feed = {}
    for l in range(4):
        w = np.asarray(inp[f"W{l}"], np.float32)
        asb = make_as_block(np.asarray(inp[f"as{l}"], np.float32))
        adb = make_as_block(np.asarray(inp[f"ad{l}"], np.float32))
        feed[f"w{l}"] = w
        feed[f"wa{l}"] = np.ascontiguousarray(
            np.concatenate([w @ asb, w @ adb], axis=1))
    for l in range(3):
        feed[f"gcol{l}"] = np.asarray(inp[f"g{l}"], np.float32).reshape(cfg.hd, 1)
        feed[f"becol{l}"] = np.asarray(inp[f"be{l}"], np.float32).reshape(cfg.hd, 1)

    xp = np.empty_like(x)
    xp[meta.perm] = x
    maps = []
    for k in range(cfg.n_cores):
        m = dict(feed)
        m["xT"] = np.ascontiguousarray(xp[k * nloc:(k + 1) * nloc].T)
        m["idxlh"] = meta.idx_lh[k]
        m["aidx"] = meta.aidx[k]
        m["doff"] = meta.doff[k]
        m["ptab"] = meta.ptab[k]
        maps.append(m)
    return maps


def finish_host(cfg: GATConfig, meta: GraphMeta, results: list[dict],
                b3: np.ndarray) -> np.ndarray:
    pooled = sum(r["pool_out"] for r in results)
    cnt = np.maximum(meta.cnt, 1.0)[:, None]
    return (pooled / cnt + np.asarray(b3, np.float32)[None, :]).astype(np.float32)


# ---------------------------------------------------------------------------


def build_bass(cfg: GATConfig, meta: GraphMeta, stub: bool = False,
               debug: bool = False):
    n, nloc, nblk, hd = cfg.n, cfg.nloc, cfg.nblk, cfg.hd
    TT = meta.total_tiles
    GDT = BF16 if cfg.bf16 else F32
    assert cfg.bf16
    ROW, ROW3 = 256, 128
    AROW = 64  # [ald(4) | pad(60)] f32
    CT = cfg.chunk_tiles
    NPAD = 64  # hcat table row padding beyond n (gather never reads them)

    nc = bacc.Bacc("TRN2", target_bir_lowering=False, debug=False,
                   num_devices=cfg.n_cores)

    xT_in = nc.dram_tensor("xT", [P, nloc], F32, kind="ExternalInput").ap()
    idxlh_in = nc.dram_tensor("idxlh", [P, TT * 8], I16, kind="ExternalInput").ap()
    aidx_in = nc.dram_tensor("aidx", [P, TT * 8], I16, kind="ExternalInput").ap()
    doff_in = nc.dram_tensor("doff", [P, TT], F32, kind="ExternalInput").ap()
    ptab_in = nc.dram_tensor("ptab", [P, nblk * 64], F32, kind="ExternalInput").ap()
    w_in, wa_in, g_in, be_in = [], [], [], []
    for l in range(4):
        din = cfg.f_in if l == 0 else hd
        dout = hd if l < 3 else cfg.out_c
        nh = cfg.heads if l < 3 else 1
        w_in.append(nc.dram_tensor(f"w{l}", [din, dout], F32, kind="ExternalInput").ap())
        wa_in.append(nc.dram_tensor(f"wa{l}", [din, 2 * nh], F32, kind="ExternalInput").ap())
    for l in range(3):
        g_in.append(nc.dram_tensor(f"gcol{l}", [hd, 1], F32, kind="ExternalInput").ap())
        be_in.append(nc.dram_tensor(f"becol{l}", [hd, 1], F32, kind="ExternalInput").ap())
    pool_out = nc.dram_tensor("pool_out", [cfg.ng, cfg.out_c], F32,
                              kind="ExternalOutput").ap()
    dbg_y, dbg_h, dbg_acc = [], [], []
    if debug:
        for l in range(4):
            row = 192 if l < 3 else 128
            dbg_h.append(nc.dram_tensor(f"dbg_h{l}", [nloc, row], F32,
                                        kind="ExternalOutput").ap())
            dbg_acc.append(nc.dram_tensor(f"dbg_acc{l}", [P, nblk * 132], F32,
                                          kind="ExternalOutput").ap())
            if l < 3:
                dbg_y.append(nc.dram_tensor(f"dbg_y{l}", [P, nloc], F32,
                                            kind="ExternalOutput").ap())

    if stub:
        with tile.TileContext(nc) as tc:
            with tc.tile_pool(name="sb", bufs=1) as pool:
                t = pool.tile([cfg.ng, cfg.out_c], F32)
                nc.vector.memset(t[:], 0.0)
                nc.sync.dma_start(out=pool_out[:], in_=t[:])
        nc.compile()
        return nc

    hcat, hloc, ald = [], [], []
    for l in range(4):
        row = ROW if l < 3 else ROW3
        hcat.append(nc.dram_tensor(f"hcat{l}", [n + NPAD, row], GDT,
                                   addr_space="Shared").ap())
        hloc.append(nc.dram_tensor(f"hloc{l}", [nloc, row], GDT).ap())
        ald.append(nc.dram_tensor(f"aldrep{l}", [nloc, AROW], F32).ap())
    st_in = nc.dram_tensor("st_in", [P, 2], F32).ap()
    st_out = nc.dram_tensor("st_out", [P, 2], F32, addr_space="Shared").ap()

    # tile index -> (region, block); first/last tile per (region, block)
    tmap = []
    for r in range(2):
        caps = meta.cap_l if r == 0 else meta.cap_h
        for b in range(nblk):
            tmap += [(r, b)] * caps[b]
    assert len(tmap) == TT
    first_of, last_of = {}, {}
    for i, rb in enumerate(tmap):
        if rb not in first_of:
            first_of[rb] = i
        last_of[rb] = i

    rg = [list(range(cfg.n_cores))]

    with tile.TileContext(nc) as tc:
        with (
            tc.tile_pool(name="persist", bufs=1) as pp,
            tc.tile_pool(name="ncol", bufs=1) as npool,
            tc.tile_pool(name="work", bufs=2) as wp,
            tc.tile_pool(name="spool", bufs=4) as sp,
            tc.tile_pool(name="psA", bufs=2, space="PSUM") as psA,
            tc.tile_pool(name="psT", bufs=2, space="PSUM") as psT,
            tc.tile_pool(name="psS", bufs=2, space="PSUM") as psS,
            tc.tile_pool(name="psP", bufs=1, space="PSUM") as psP,
        ):
            idxlh = pp.tile([P, TT * 8], I16)
            nc.sync.dma_start(out=idxlh[:], in_=idxlh_in[:])
            aidx = pp.tile([P, TT * 8], I16)
            nc.sync.dma_start(out=aidx[:], in_=aidx_in[:])
            ptab = pp.tile([P, nblk * 64], F32)
            nc.sync.dma_start(out=ptab[:], in_=ptab_in[:])
            ident = pp.tile([P, P], F32)
            make_identity(nc, ident[:])
            doff = pp.tile([P, TT], F32)
            nc.sync.dma_start(out=doff[:], in_=doff_in[:])
            iota = pp.tile([P, P], BF16)
            nc.gpsimd.iota(iota[:], pattern=[[1, P]], base=0, channel_multiplier=0,
                           allow_small_or_imprecise_dtypes=True)
            acc = pp.tile([P, nblk * 132], F32)

            yT = npool.tile([P, nloc], F32)
            scr = npool.tile([P, nloc], F32)
            nc.sync.dma_start(out=yT[:], in_=xT_in[:])

            for l in range(4):
                row = ROW if l < 3 else ROW3
                nh = cfg.heads if l < 3 else 1
                dout = hd if l < 3 else cfg.out_c
                ch = dout // nh
                ccols = dout + nh

                # ================= transform: yT -> hcat[l], ald[l] ========
                wl = sp.tile([P, dout], F32, tag="wl")
                nc.sync.dma_start(out=wl[:], in_=w_in[l][:])
                wal = sp.tile([P, 2 * nh], F32, tag="wal")
                nc.sync.dma_start(out=wal[:], in_=wa_in[l][:])
                nchunks = (nloc + 511) // 512
                for ci in range(nchunks):
                    c0 = ci * 512
                    cw = min(512, nloc - c0)
                    ph = psT.tile([P, 512], F32, space="PSUM", tag="ph")
                    nc.tensor.matmul(ph[:dout, :cw], lhsT=wl[:], rhs=yT[:, c0:c0 + cw],
                                     start=True, stop=True)
                    hT_sb = wp.tile([P, 512], F32, tag="hT_sb")
                    nc.vector.tensor_copy(out=hT_sb[:dout, :cw], in_=ph[:dout, :cw])
                    pa = psT.tile([P, 512], F32, space="PSUM", tag="ph")
                    nc.tensor.matmul(pa[:2 * nh, :cw], lhsT=wal[:], rhs=yT[:, c0:c0 + cw],
                                     start=True, stop=True)
                    al_sb = wp.tile([8, 512], F32, tag="al_sb")
                    nc.vector.tensor_copy(out=al_sb[:2 * nh, :cw], in_=pa[:2 * nh, :cw])

                    for bi in range(4):
                        b = ci * 4 + bi
                        if b >= nblk or b * P >= c0 + cw:
                            break
                        o0 = bi * P
                        bw = min(P, nloc - b * P)
                        # node-major via matmul with identity: out = in.T @ I
                        pt = psS.tile([P, P + 8], F32, space="PSUM", tag="pt")
                        nc.tensor.matmul(pt[:bw, 0:dout],
                                         lhsT=hT_sb[:dout, o0:o0 + bw],
                                         rhs=ident[:dout, 0:dout],
                                         start=True, stop=True)
                        nc.tensor.matmul(pt[:bw, P:P + 2 * nh],
                                         lhsT=al_sb[:2 * nh, o0:o0 + bw],
                                         rhs=ident[:2 * nh, 0:2 * nh],
                                         start=True, stop=True)
                        stg = wp.tile([P, row], GDT, tag="stg")
                        nc.vector.memset(stg[:, dout + nh:row], 0.0)
                        nc.vector.tensor_copy(out=stg[:bw, 0:dout], in_=pt[:bw, 0:dout])
                        nc.vector.tensor_copy(out=stg[:bw, dout:dout + nh],
                                              in_=pt[:bw, P:P + nh])
                        nc.sync.dma_start(out=hloc[l][b * P:b * P + bw, :],
                                          in_=stg[:bw, :])
                        stga = wp.tile([P, AROW], F32, tag="stga")
                        nc.vector.memset(stga[:, nh:AROW], 0.0)
                        nc.vector.tensor_copy(out=stga[:bw, 0:nh],
                                              in_=pt[:bw, P + nh:P + 2 * nh])
                        nc.sync.dma_start(out=ald[l][b * P:b * P + bw, :],
                                          in_=stga[:bw, :])

                if "nocc" not in cfg.ablate:
                    nc.gpsimd.collective_compute(
                        "AllGather", ALU.bypass, replica_groups=rg,
                        ins=[hloc[l][:, :].opt()],
                        outs=[hcat[l][0:n, :].opt()],
                    )

                # ================= aggregation =============================
                t = 0
                psum_cur = None
                while t < TT:
                    tn = min(CT, TT - t)
                    r0 = tmap[t][0]
                    for i in range(1, tn):
                        if tmap[t + i][0] != r0:
                            tn = i
                            break
                    G = wp.tile([P, CT, row], GDT, tag="G", bufs=cfg.gbufs)
                    tbl = (hcat[l][0:cfg.half, :] if r0 == 0
                           else hcat[l][cfg.half:n, :])
                    if "nohcat" not in cfg.ablate:
                        nc.gpsimd.dma_gather(
                            out_ap=G[:, :tn, :], in_ap=tbl,
                            idxs_ap=idxlh[:, t * 8:(t + tn) * 8],
                            num_idxs=tn * P, num_idxs_reg=tn * P, elem_size=row,
                            single_packet=False)
                    A = wp.tile([P, CT, AROW], F32, tag="A", bufs=cfg.gbufs)
                    if "noald" not in cfg.ablate:
                        nc.gpsimd.dma_gather(
                            out_ap=A[:, :tn, :], in_ap=ald[l][0:nloc, :],
                            idxs_ap=aidx[:, t * 8:(t + tn) * 8],
                            num_idxs=tn * P, num_idxs_reg=tn * P, elem_size=AROW,
                            single_packet=False)

                    ecols = G[:, :tn, dout:dout + nh]
                    if "noe" not in cfg.ablate:
                        EX = wp.tile([P, CT, 8], F32, tag="EX", bufs=cfg.gbufs)
                        ef = EX[:, :tn, 0:nh]
                        ef2 = EX[:, :tn, nh:2 * nh]
                        nc.vector.tensor_tensor(out=ef, in0=ecols,
                                                in1=A[:, :tn, 0:nh], op=ALU.add)
                        # leaky_relu(z, 0.2) = max(z, 0.2z) (HW Lrelu ignores alpha)
                        nc.vector.tensor_scalar(out=ef2, in0=ef, scalar1=0.2,
                                                scalar2=None, op0=ALU.mult)
                        nc.vector.tensor_tensor(out=ef, in0=ef, in1=ef2, op=ALU.max)
                        nc.scalar.activation(out=ecols, in_=ef, func=AF.Exp)
                        # duplicated-pair copy of ex so the msg multiply can run
                        # in the DVE 2x_1p packed-bf16 mode (last dim [1,2])
                        XD = wp.tile([P, CT, 2 * nh], BF16, tag="XD", bufs=cfg.gbufs)
                        nc.scalar.activation(
                            out=XD[:, :tn, :],
                            in_=ef.unsqueeze(3).to_broadcast([P, tn, nh, 2]),
                            func=AF.Exp)
                    if "nomsg" not in cfg.ablate:
                        exb = (XD[:, :tn, :]
                               .rearrange("p t (h two) -> p t h two", two=2)
                               .unsqueeze(3)
                               .to_broadcast([P, tn, nh, ch // 2, 2]))
                        msg = G[:, :tn, 0:dout].rearrange(
                            "p t (h c two) -> p t h c two", h=nh, two=2)
                        nc.vector.tensor_tensor(out=msg, in0=msg, in1=exb, op=ALU.mult)

                    for i in range(tn):
                        r, b = tmap[t + i]
                        first = (t + i) == first_of[(r, b)]
                        last = (t + i) == last_of[(r, b)]
                        if first:
                            psum_cur = psA.tile([P, ccols], F32, space="PSUM",
                                                tag="agg")
                        S = sp.tile([P, P], GDT, tag="S")
                        if "nos" not in cfg.ablate:
                            nc.vector.tensor_scalar(
                                out=S[:], in0=iota[:],
                                scalar1=doff[:, t + i:t + i + 1], scalar2=None,
                                op0=ALU.is_equal)
                        if "nomm" not in cfg.ablate:
                            nc.tensor.matmul(psum_cur[:], lhsT=S[:],
                                             rhs=G[:, i, 0:ccols],
                                             start=first, stop=last,
                                             skip_group_check=(l == 3))
                        if last and "nomm" not in cfg.ablate:
                            c0a = b * 132
                            if r == 0:
                                nc.vector.tensor_copy(out=acc[:, c0a:c0a + ccols],
                                                      in_=psum_cur[:])
                            else:
                                nc.vector.tensor_tensor(out=acc[:, c0a:c0a + ccols],
                                                        in0=acc[:, c0a:c0a + ccols],
                                                        in1=psum_cur[:], op=ALU.add)
                    t += tn

                if debug:
                    nc.sync.dma_start(out=dbg_h[l][:, :], in_=hloc[l][:, :])
                    dacc = wp.tile([P, nblk * 132], F32, tag="dacc")
                    nc.vector.tensor_copy(out=dacc[:], in_=acc[:])
                    nc.sync.dma_start(out=dbg_acc[l][:, :], in_=dacc[:])

                # ================= post-aggregation ========================
                if l < 3:
                    for b in range(nblk):
                        bw = min(P, nloc - b * P)
                        c0a = b * 132
                        rec = sp.tile([P, nh], F32, tag="rec")
                        nc.vector.tensor_scalar(
                            out=rec[:], in0=acc[:, c0a + dout:c0a + dout + nh],
                            scalar1=cfg.eps_sm, scalar2=None, op0=ALU.add)
                        nc.vector.reciprocal(out=rec[:], in_=rec[:])
                        zb = wp.tile([P, P], F32, tag="zb")
                        nc.vector.tensor_tensor(
                            out=zb[:].rearrange("p (h c) -> p h c", h=nh),
                            in0=acc[:, c0a:c0a + dout].rearrange("p (h c) -> p h c", h=nh),
                            in1=rec[:].unsqueeze(2).to_broadcast([P, nh, ch]),
                            op=ALU.mult)
                        pz = psS.tile([P, P + 8], F32, space="PSUM", tag="pt")
                        nc.tensor.matmul(pz[:, 0:P], lhsT=zb[:], rhs=ident[:],
                                         start=True, stop=True)
                        nc.vector.tensor_copy(out=scr[:, b * P:b * P + bw],
                                              in_=pz[:, 0:bw])
                    stats = sp.tile([P, 2], F32, tag="stats")
                    nc.vector.tensor_reduce(out=stats[:, 0:1], in_=scr[:, 0:nloc],
                                            axis=mybir.AxisListType.X, op=ALU.add)
                    nc.scalar.activation(out=yT[:, 0:nloc], in_=scr[:, 0:nloc],
                                         func=AF.Square)
                    nc.vector.tensor_reduce(out=stats[:, 1:2], in_=yT[:, 0:nloc],
                                            axis=mybir.AxisListType.X, op=ALU.add)
                    nc.sync.dma_start(out=st_in[:], in_=stats[:])
                    nc.gpsimd.collective_compute(
                        "AllReduce", ALU.add, replica_groups=rg,
                        ins=[st_in[:, :].opt()], outs=[st_out[:, :].opt()])
                    gst = sp.tile([P, 2], F32, tag="gst")
                    nc.sync.dma_start(out=gst[:], in_=st_out[:])

                    mu = sp.tile([P, 4], F32, tag="mu")  # [mu | var | rst | tmp]
                    nc.vector.tensor_scalar(out=mu[:, 0:2], in0=gst[:, 0:2],
                                            scalar1=1.0 / n, scalar2=None,
                                            op0=ALU.mult)
                    nc.vector.tensor_tensor(out=mu[:, 3:4], in0=mu[:, 0:1],
                                            in1=mu[:, 0:1], op=ALU.mult)
                    nc.vector.tensor_tensor(out=mu[:, 1:2], in0=mu[:, 1:2],
                                            in1=mu[:, 3:4], op=ALU.subtract)
                    nc.vector.tensor_scalar(out=mu[:, 1:2], in0=mu[:, 1:2],
                                            scalar1=cfg.eps_bn, scalar2=None,
                                            op0=ALU.add)
                    nc.scalar.activation(out=mu[:, 2:3], in_=mu[:, 1:2], func=AF.Sqrt)
                    nc.vector.reciprocal(out=mu[:, 2:3], in_=mu[:, 2:3])
                    # Newton: r = r*(1.5 - 0.5*var*r^2)
                    nc.vector.tensor_tensor(out=mu[:, 3:4], in0=mu[:, 2:3],
                                            in1=mu[:, 2:3], op=ALU.mult)
                    nc.vector.tensor_tensor(out=mu[:, 3:4], in0=mu[:, 3:4],
                                            in1=mu[:, 1:2], op=ALU.mult)
                    nc.vector.tensor_scalar(out=mu[:, 3:4], in0=mu[:, 3:4],
                                            scalar1=-0.5, scalar2=1.5,
                                            op0=ALU.mult, op1=ALU.add)
                    nc.vector.tensor_tensor(out=mu[:, 2:3], in0=mu[:, 2:3],
                                            in1=mu[:, 3:4], op=ALU.mult)
                    ab = sp.tile([P, 2], F32, tag="ab")  # [A | B]
                    gc = sp.tile([P, 2], F32, tag="gc")
                    nc.sync.dma_start(out=gc[:, 0:1], in_=g_in[l][:])
                    nc.sync.dma_start(out=gc[:, 1:2], in_=be_in[l][:])
                    nc.vector.tensor_tensor(out=ab[:, 0:1], in0=gc[:, 0:1],
                                            in1=mu[:, 2:3], op=ALU.mult)
                    nc.vector.tensor_tensor(out=ab[:, 1:2], in0=mu[:, 0:1],
                                            in1=ab[:, 0:1], op=ALU.mult)
                    nc.vector.tensor_tensor(out=ab[:, 1:2], in0=gc[:, 1:2],
                                            in1=ab[:, 1:2], op=ALU.subtract)
                    # y = z*A + B ; elu(y) = max(y,0) + exp(min(y,0)) - 1
                    nc.vector.tensor_scalar(out=yT[:, 0:nloc], in0=scr[:, 0:nloc],
                                            scalar1=ab[:, 0:1], scalar2=ab[:, 1:2],
                                            op0=ALU.mult, op1=ALU.add)
                    nc.vector.tensor_scalar(out=scr[:, 0:nloc], in0=yT[:, 0:nloc],
                                            scalar1=0.0, scalar2=None, op0=ALU.min)
                    nc.scalar.activation(out=scr[:, 0:nloc], in_=scr[:, 0:nloc],
                                         func=AF.Exp)
                    nc.vector.tensor_scalar(out=yT[:, 0:nloc], in0=yT[:, 0:nloc],
                                            scalar1=0.0, scalar2=None, op0=ALU.max)
                    nc.vector.tensor_tensor(out=yT[:, 0:nloc], in0=yT[:, 0:nloc],
                                            in1=scr[:, 0:nloc], op=ALU.add)
                    nc.vector.tensor_scalar(out=yT[:, 0:nloc], in0=yT[:, 0:nloc],
                                            scalar1=-1.0, scalar2=None, op0=ALU.add)
                else:
                    pool_ps = psP.tile([cfg.ng, cfg.out_c], F32, space="PSUM")
                    for b in range(nblk):
                        c0a = b * 132
                        rec = sp.tile([P, 1], F32, tag="rec")
                        nc.vector.tensor_scalar(
                            out=rec[:], in0=acc[:, c0a + dout:c0a + dout + 1],
                            scalar1=cfg.eps_sm, scalar2=None, op0=ALU.add)
                        nc.vector.reciprocal(out=rec[:], in_=rec[:])
                        hb = wp.tile([P, cfg.out_c], F32, tag="hb")
                        nc.vector.tensor_scalar(out=hb[:], in0=acc[:, c0a:c0a + dout],
                                                scalar1=rec[:, 0:1], scalar2=None,
                                                op0=ALU.mult)
                        nc.tensor.matmul(pool_ps[:], lhsT=ptab[:, b * 64:(b + 1) * 64],
                                         rhs=hb[:], start=(b == 0),
                                         stop=(b == nblk - 1), skip_group_check=True)
                    po = wp.tile([cfg.ng, cfg.out_c], F32, tag="po")
                    nc.vector.tensor_copy(out=po[:], in_=pool_ps[:])
                    nc.sync.dma_start(out=pool_out[:], in_=po[:])

    nc.compile()
    return nc


# ---------------------------------------------------------------------------
# Runner: persistent jitted executable over the 8 axon-tunneled NeuronCores
# ---------------------------------------------------------------------------

class PersistentRunner:
    """Like run_bass_kernel_spmd's axon path, but keeps the jitted callable
    and device-resident inputs alive so repeated calls only pay dispatch +
    execute (used for timing; also makes repeated kernel() calls fast)."""

    def __init__(self, nc, in_maps):
        import jax
        import numpy as _np
        from jax.sharding import Mesh, PartitionSpec
        from jax.experimental.shard_map import shard_map
        import concourse.mybir as _mybir
        from concourse import bass2jax as _b2j

        _b2j.install_neuronx_cc_hook()
        self.jax = jax
        n_cores = len(in_maps)
        nc_mod = nc.m
        partition_name = (nc.partition_id_tensor.name
                          if nc.partition_id_tensor else None)
        in_names, out_names, out_avals, zero_outs = [], [], [], []
        for alloc in nc_mod.functions[0].allocations:
            if not isinstance(alloc, _mybir.MemoryLocationSet):
                continue
            name = alloc.memorylocations[0].name
            if alloc.kind == "ExternalInput":
                if name != partition_name:
                    in_names.append(name)
            elif alloc.kind == "ExternalOutput":
                shape = tuple(alloc.tensor_shape)
                dtype = _mybir.dt.np(alloc.dtype)
                out_names.append(name)
                out_avals.append(jax.core.ShapedArray(shape, dtype))
                zero_outs.append(_np.zeros(shape, dtype))
        n_params = len(in_names)
        n_outs = len(out_avals)
        all_in_names = list(in_names) + list(out_names)
        if partition_name is not None:
            all_in_names.append(partition_name)
        self.out_names = out_names
        self.out_avals = out_avals
        self.n_cores = n_cores

        def _body(*args):
            operands = list(args)
            if partition_name is not None:
                operands.append(_b2j.partition_id_tensor())
            outs = _b2j._bass_exec_p.bind(
                *operands,
                out_avals=tuple(out_avals),
                in_names=tuple(all_in_names),
                out_names=tuple(out_names),
                lowering_input_output_aliases=(),
                sim_require_finite=True,
                sim_require_nnan=True,
                nc=nc,
            )
            return tuple(outs)

        devices = jax.devices()[:n_cores]
        assert len(devices) == n_cores
        mesh = Mesh(_np.asarray(devices), ("core",))
        in_specs = (PartitionSpec("core"),) * (n_params + n_outs)
        out_specs = (PartitionSpec("core"),) * n_outs
        donate = tuple(range(n_params, n_params + n_outs))
        self._fn = jax.jit(
            shard_map(_body, mesh=mesh, in_specs=in_specs, out_specs=out_specs,
                      check_rep=False),
            donate_argnums=donate, keep_unused=True)
        concat_in = [
            _np.concatenate([_np.asarray(in_maps[c][name]) for c in range(n_cores)],
                            axis=0)
            for name in in_names
        ]
        self._dev_in = [jax.device_put(a) for a in concat_in]
        self._zero_outs = zero_outs

    def __call__(self):
        import numpy as _np
        zeros = [_np.zeros((self.n_cores * z.shape[0], *z.shape[1:]), z.dtype)
                 for z in self._zero_outs]
        outs = self._fn(*self._dev_in, *zeros)
        self.jax.block_until_ready(outs)
        return [
            {name: _np.asarray(outs[i]).reshape(self.n_cores, *self.out_avals[i].shape)[c]
             for i, name in enumerate(self.out_names)}
            for c in range(self.n_cores)
        ]

    def time_ns(self, iters=10):
        import time as _t
        self()  # warm
        samples = []
        for _ in range(iters):
            t0 = _t.perf_counter_ns()
            self()
            samples.append(_t.perf_counter_ns() - t0)
        samples.sort()
        return samples[len(samples) // 2]


def _capture_makespan():
    """Attach a log handler that captures the Tile scheduling-sim makespan."""
    import logging, re
    records = []

    class _H(logging.Handler):
        def emit(self, r):
            m = re.search(r"Simulation completed at time (\d+)", r.getMessage())
            if m:
                records.append(int(m.group(1)))

    h = _H()
    h.setLevel(logging.DEBUG)
    for name in ("concourse", "concourse.bass_interp"):
        lg = logging.getLogger(name)
        lg.addHandler(h)
        lg.setLevel(logging.DEBUG)
    return records, h


def _release_capture(h):
    import logging
    for name in ("concourse", "concourse.bass_interp"):
        logging.getLogger(name).removeHandler(h)


_CACHE = {}


def _get_runner(inputs, stub=False):
    import hashlib
    cfg = GATConfig()
    ei = np.asarray(inputs["edge_index"], np.int64)
    bi = np.asarray(inputs["batch_ids"], np.int64)
    key = (hashlib.sha1(ei.tobytes()).hexdigest()[:16],
           hashlib.sha1(bi.tobytes()).hexdigest()[:16], stub)
    ent = _CACHE.get(key)
    if ent is None:
        meta = preprocess(cfg, ei, bi)
        records, h = _capture_makespan()
        nc = build_bass(cfg, meta, stub=stub)
        _release_capture(h)
        ent = {"cfg": cfg, "meta": meta, "nc": nc, "runner": None, "sig": None,
               "makespan_ns": max(records) if records else None}
        _CACHE[key] = ent
    maps = build_inputs(ent["cfg"], ent["meta"], inputs)
    sig = tuple(hash(np.asarray(inputs[k], np.float32).tobytes())
                for k in ("x", "W0", "W1", "W2", "W3"))
    if ent["runner"] is None or ent["sig"] != sig:
        ent["runner"] = PersistentRunner(ent["nc"], maps)
        ent["sig"] = sig
    return ent


def kernel(**inputs) -> np.ndarray:
    """Full GAT forward on 8 NeuronCores. Takes the full unsharded inputs of
    reference.setup_inputs(); returns the [64, 64] float32 pooled output."""
    ent = _get_runner(inputs)
    results = ent["runner"]()
    return finish_host(ent["cfg"], ent["meta"], results, inputs["b3"])

